# revision 16
# baseline (speedup 1.0000x reference)
"""Additive (Bahdanau) attention on 8 TRN2 NeuronCores — V4.

Reference computation:
    qp = queries @ W_q                  (bs, n_q, 64)
    kp = keys @ W_k                     (bs, n_k, 64)
    scores[b,q,k] = sum_h w_v[h] * tanh(qp[b,q,h] + kp[b,k,h])
    out = softmax(scores, -1) @ values

tanh(x) ~= sum_{j<4} c_j sin((2j+1) w0 x): weighted-LS fit under the
empirical arg distribution (~N(0, sqrt2), |arg| <= 8.8), w0=0.336.
Angle addition makes the scores separable into matmuls with contraction
2*64 per harmonic. The HW Sin is a polynomial valid only on |x| <~ pi
(measured: err 6e-8 below 3, diverges after), so j>=1 args are range-
reduced exactly as V2: z = x/2pi + 36.xx (turns; packing phase in the
per-partition fraction), fp32 bit surgery keeps frac(z) (low 18 mantissa
bits, OR exponent 127 -> v in [1, 1.03125]), then Sin(-64pi v + 65pi)
lands in (-pi, pi].

Vs V2 (43.6us): J=4 refit (better e2e error than V2's J=5 minimax fit),
host-side transposed bf16 inputs feed the projections directly (no PE
transposes/casts/identity), v pre-tiled bf16, output bf16. Input DMAs
split across the idle sync/vector queues (a single DMA engine moves only
~90GB/s; parallel queues parallelize engines). j=0,1 banks bf16, (2,3)
packed fp8e4 DoubleRow. Exp on ScalarE; qp2 PSUM->SBUF copy on ScalarE
(DVE is the trig-phase bottleneck).

Sharding: fully data-parallel, no collectives. Core c handles batch c//2,
query half c%2: (512 q, 1024 k).
"""

import numpy as np

BS, NQ, NK = 4, 1024, 1024
QD, KD, VD, HID = 128, 128, 128, 64
NCORES = 8
NQH = NQ // 2  # queries per core

J = 4
W0 = 0.3360
FOURIER_C = [1.2083691, 0.29709556, 0.08487929, 0.04759338]
HALF_PI = 1.5707963267948966
TWO_PI = 6.283185307179586
PI64 = 64 * 3.141592653589793

N_DVE_EXP = 2  # tiles of exp computed on DVE (Schraudolph); rest ScalarE

_CACHED = {}


def _build():
    import concourse.bacc as bacc
    import concourse.mybir as mybir
    from concourse import tile
    from concourse.alu_op_type import AluOpType

    F32 = mybir.dt.float32
    U32 = mybir.dt.uint32
    BF16 = mybir.dt.bfloat16
    FP8 = mybir.dt.float8e4
    A = mybir.ActivationFunctionType
    DR = mybir.MatmulPerfMode.DoubleRow

    nc = bacc.Bacc(None, target_bir_lowering=False)

    qT = nc.declare_dram_parameter("qT", [QD, NQH], BF16, isOutput=False)
    kT = nc.declare_dram_parameter("kT", [KD, NK], BF16, isOutput=False)
    v4 = nc.declare_dram_parameter("v4", [128, 8, 128], BF16, isOutput=False)
    wqk = nc.declare_dram_parameter("wqk", [128, 256], BF16, isOutput=False)
    cvec = nc.declare_dram_parameter("cvec", [128, 16], F32, isOutput=False)
    outT = nc.declare_dram_parameter("outT", [128, NQH], BF16, isOutput=True)
    sums = nc.declare_dram_parameter("sums", [1, NQH], F32, isOutput=True)

    NQC = NQH // 128  # 4 query chunks
    NKC = NK // 128   # 8 key chunks
    S1 = [float((2 * j + 1) * W0 / TWO_PI) for j in range(J)]

    with tile.TileContext(nc) as tc:
        with (
            tc.tile_pool(name="consts", bufs=1) as consts,
            tc.tile_pool(name="io", bufs=1) as io,
            tc.tile_pool(name="work", bufs=4) as work,
            tc.tile_pool(name="jb", bufs=3) as jb,
            tc.tile_pool(name="sm", bufs=NKC) as sm,
            tc.tile_pool(name="ps", bufs=8, space="PSUM") as ps,
        ):
            # ---- input DMAs. k gates everything: its halves go first on
            # sync and scalar (descgen overlaps the scalar
            # table loads) so two DMA engines run in parallel; q follows
            # on sync. v/wqk/cvec on gpsimd.
            kT_sb = io.tile([KD, NK], BF16, tag="kT")
            qT_sb = io.tile([QD, NQH], BF16, tag="qT")
            wqk_sb = consts.tile([128, 256], BF16, tag="wqk")
            cvec_sb = consts.tile([128, 16], F32, tag="cvec")
            nc.sync.dma_start(kT_sb[:, 0:256], kT[:, 0:256])
            nc.scalar.dma_start(kT_sb[:, 256:512], kT[:, 256:512])
            nc.sync.dma_start(kT_sb[:, 512:768], kT[:, 512:768])
            nc.scalar.dma_start(kT_sb[:, 768:1024], kT[:, 768:1024])
            nc.gpsimd.dma_start(wqk_sb[:], wqk[:, :])
            nc.gpsimd.dma_start(cvec_sb[:], cvec[:, :])
            nc.sync.dma_start(qT_sb[:, 0:256], qT[:, 0:256])
            nc.gpsimd.dma_start(qT_sb[:, 256:512], qT[:, 256:512])
            sphq = cvec_sb[:, 0:1]
            sphk = cvec_sb[:, 1:2]
            biasq = cvec_sb[:, 2:3]
            biask = cvec_sb[:, 3:4]
            bias65 = cvec_sb[:, 4:5]

            ones16 = consts.tile([128, 1], BF16, tag="ones16")
            nc.gpsimd.memset(ones16[:], 1.0)



            # ---- projections (bf16, PE) ----
            wq_w = wqk_sb[:, 0:128]
            wk_w = wqk_sb[:, 128:256]
            pk0 = ps.tile([128, 512], F32, tag="t512", name="pk0")
            pk1 = ps.tile([128, 512], F32, tag="t512", name="pk1")
            pq = ps.tile([128, 512], F32, tag="t512", name="pq")
            nc.tensor.matmul(pk0[:], wk_w, kT_sb[:, 0:512], start=True, stop=True)
            nc.tensor.matmul(pk1[:], wk_w, kT_sb[:, 512:1024], start=True, stop=True)
            nc.tensor.matmul(pq[:], wq_w, qT_sb[:], start=True, stop=True)

            # kp to SBUF for the j>=1 preps: halves on DVE; qp2 on ScalarE
            # (keeps the DVE trig chain shorter).
            kp2 = io.tile([128, NK], F32, tag="kp2")
            qp2 = io.tile([128, NQH], F32, tag="qp2")
            nc.vector.tensor_copy(kp2[:, 0:512], pk0[:])
            nc.vector.tensor_copy(kp2[:, 512:1024], pk1[:])
            nc.vector.tensor_copy(qp2[:], pq[:])

            # ---- trig banks ----
            # K rows [cos_j | sin_j] unscaled; Q rows [sin_j | cos_j] * c_j w_v.
            # j=0,1 bf16; (2,3) fp8e4 packed for one DoubleRow pass.
            ksb = {j: jb.tile([128, NK], BF16, tag="ks", name=f"ks{j}")
                   for j in (0, 1)}
            sqb = {j: jb.tile([128, NQH], BF16, tag="sq", name=f"sq{j}")
                   for j in (0, 1)}
            kspair = jb.tile([128, 2, NK], FP8, tag="kspair")
            sqpair = jb.tile([128, 2, NQH], FP8, tag="sqpair")
            sqf = {j: work.tile([128, NQH], BF16, tag="sqf", name=f"sqf{j}",
                                bufs=4) for j in range(J)}

            def ks_dst(j):
                return kspair[:, j - 2, :] if j >= 2 else ksb[j][:]

            def sq_dst(j):
                return sqpair[:, j - 2, :] if j >= 2 else sqb[j][:]

            def scale_q(j):
                nc.vector.tensor_scalar_mul(sq_dst(j), sqf[j][:],
                                            cvec_sb[:, 5 + j:6 + j])

            # j=0 sins straight off the projection PSUMs.
            nc.scalar.activation(ksb[0][:, 0:512], pk0[:], A.Sin,
                                 bias=biask, scale=W0)
            nc.scalar.activation(ksb[0][:, 512:1024], pk1[:], A.Sin,
                                 bias=biask, scale=W0)
            nc.scalar.activation(sqf[0][:], pq[:], A.Sin,
                                 bias=biasq, scale=W0)
            scale_q(0)

            # j>=1: range-reduce on DVE (z in [32,64) turns incl. packing
            # phase; frac via bit surgery), Sin on ScalarE. Interleave the
            # q scales behind the matching sins.
            zk = {j: work.tile([128, NK], F32, tag="zk", name=f"zk{j}",
                               bufs=2) for j in (1, 2, 3)}
            vk = {j: work.tile([128, NK], F32, tag="vk", name=f"vk{j}",
                               bufs=3) for j in (1, 2, 3)}
            zq = {j: work.tile([128, NQH], F32, tag="zq", name=f"zq{j}",
                               bufs=2) for j in (1, 2, 3)}
            vq = {j: work.tile([128, NQH], F32, tag="vq", name=f"vq{j}",
                               bufs=3) for j in (1, 2, 3)}

            for j in (1, 2, 3):
                nc.vector.tensor_scalar(zq[j][:], qp2[:], S1[j], sphq,
                                        AluOpType.mult, AluOpType.add)
                nc.vector.tensor_scalar(vq[j][:].bitcast(U32),
                                        zq[j][:].bitcast(U32),
                                        0x0003FFFF, 0x3F800000,
                                        AluOpType.bitwise_and,
                                        AluOpType.bitwise_or)
                nc.scalar.activation(sqf[j][:], vq[j][:], A.Sin,
                                     scale=-PI64, bias=bias65)
                scale_q(j)
                nc.vector.tensor_scalar(zk[j][:], kp2[:], S1[j], sphk,
                                        AluOpType.mult, AluOpType.add)
                nc.vector.tensor_scalar(vk[j][:].bitcast(U32),
                                        zk[j][:].bitcast(U32),
                                        0x0003FFFF, 0x3F800000,
                                        AluOpType.bitwise_and,
                                        AluOpType.bitwise_or)
                kd = ks_dst(j)
                nc.scalar.activation(kd[:, 0:512], vk[j][:, 0:512], A.Sin,
                                     scale=-PI64, bias=bias65)
                nc.scalar.activation(kd[:, 512:1024], vk[j][:, 512:1024],
                                     A.Sin, scale=-PI64, bias=bias65)
                if j == 1:
                    v_sb = io.tile([128, 8, 128], BF16, tag="v4")
                    nc.gpsimd.dma_start(v_sb[:], v4[:, :, :])


            # ---- score matmuls: psT[kt] = scores^T accumulated over j ----
            psT = [ps.tile([128, 512], F32, tag="t512", name=f"psT_{kt}")
                   for kt in range(NKC)]
            # p-state warmers: garbage matmuls into not-yet-started psT
            # banks (g0's start=True overwrites). The PE downclocks to
            # 1.2GHz when idle; these bridge the projections->g0 gap so
            # the score matmuls run at 2.4GHz.
            for w in range(6):
                nc.tensor.matmul(psT[w % 2][:], wqk_sb[:, 0:128],
                                 kT_sb[:, 0:512], start=True, stop=True)
            for kt in range(NKC):
                nc.tensor.matmul(psT[kt][:], ksb[0][:, kt * 128:(kt + 1) * 128],
                                 sqb[0][:], start=True, stop=False)
            for kt in range(NKC):
                nc.tensor.matmul(psT[kt][:], ksb[1][:, kt * 128:(kt + 1) * 128],
                                 sqb[1][:], start=False, stop=False)
            for kt in range(NKC // 2):
                nc.tensor.matmul(psT[kt][:],
                                 kspair[:, :, kt * 128:(kt + 1) * 128],
                                 sqpair[:], start=False, stop=True,
                                 perf_mode=DR)
            for kt in range(NKC // 2, NKC):
                nc.tensor.matmul(psT[kt][:],
                                 kspair[:, :, kt * 128:(kt + 1) * 128],
                                 sqpair[:], start=False, stop=True,
                                 perf_mode=DR)

            # ---- exp + denominators + output matmuls ----
            # First N_DVE_EXP tiles via the Schraudolph bf16 bit trick on
            # DVE (covers the ScalarE act-table reload); rest ScalarE Exp.
            I16 = mybir.dt.int16
            expT = []
            for kt in range(NKC):
                et = sm.tile([128, 512], BF16, tag="expT", name=f"expT_{kt}")
                if kt < N_DVE_EXP:
                    tf = work.tile([128, 512], F32, tag="schr", bufs=2,
                                   name=f"schr_{kt}")
                    # bits = s*2^7/ln2 + (127 - c)*2^7  (c~0.0455 incl
                    # trunc-vs-round slack), then f32->i16 convert writes
                    # the bf16 pattern of e^s.
                    nc.vector.tensor_scalar(tf[:], psT[kt][:],
                                            184.6650, 16250.18,
                                            AluOpType.mult, AluOpType.add)
                    nc.vector.tensor_copy(et[:].bitcast(I16), tf[:])
                else:
                    nc.scalar.activation(et[:], psT[kt][:], A.Exp)
                expT.append(et)

            ps_outT = ps.tile([128, 512], F32, tag="t512", name="ps_outT")
            psum_sums = ps.tile([1, 512], F32, tag="t512", name="psum_sums")
            for kt in range(NKC):
                nc.tensor.matmul(ps_outT[:], v_sb[:, kt, :], expT[kt][:],
                                 start=(kt == 0), stop=(kt == NKC - 1))
                nc.tensor.matmul(psum_sums[:], ones16[:], expT[kt][:],
                                 start=(kt == 0), stop=(kt == NKC - 1))
            sums_sb = sm.tile([1, 512], F32, tag="sums_sb")
            nc.scalar.activation(sums_sb[:], psum_sums[:], A.Copy)
            nc.gpsimd.dma_start(sums[:, :], sums_sb[:])
            outT_sb = sm.tile([128, 512], BF16, tag="outT_sb")
            nc.vector.tensor_copy(outT_sb[:, 0:256], ps_outT[:, 0:256])
            nc.sync.dma_start(outT[:, 0:256], outT_sb[:, 0:256])
            nc.vector.tensor_copy(outT_sb[:, 256:512], ps_outT[:, 256:512])
            nc.scalar.dma_start(outT[:, 256:512], outT_sb[:, 256:512])

    nc.finalize()
    return nc


def _get_nc():
    if "nc" not in _CACHED:
        _CACHED["nc"] = _build()
    return _CACHED["nc"]


def _make_consts(W_q, W_k, w_v):
    import ml_dtypes
    # wqk layout: [:, 0:128] = [W_q | W_q], [:, 128:256] = [W_k | W_k]
    wqk = np.zeros((128, 256), np.float32)
    wqk[:, 0:64] = W_q
    wqk[:, 64:128] = W_q
    wqk[:, 128:192] = W_k
    wqk[:, 192:256] = W_k
    cvec = np.zeros((128, 16), np.float32)
    # wrap-phase consts (turns, +36 so z lands in [32, 64)):
    # Q packing [sin | cos], K packing [cos | sin]
    cvec[:64, 0] = 36.0
    cvec[64:, 0] = 36.25   # sphq
    cvec[:64, 1] = 36.25
    cvec[64:, 1] = 36.0    # sphk
    cvec[64:, 2] = HALF_PI  # biasq (radians, j=0 direct)
    cvec[:64, 3] = HALF_PI  # biask
    cvec[:, 4] = 65 * np.pi  # bias65
    for j in range(J):
        cwj = (FOURIER_C[j] * w_v).astype(np.float32)
        cvec[:64, 5 + j] = cwj
        cvec[64:, 5 + j] = cwj
    return wqk.astype(ml_dtypes.bfloat16), cvec


def kernel(queries, keys, values, W_q, W_k, w_v, _trace=False, _trace_kwargs=None):
    import ml_dtypes
    from concourse.bass_utils import run_bass_kernel_spmd

    BF = ml_dtypes.bfloat16
    nc = _get_nc()
    wqk, cvec = _make_consts(
        np.asarray(W_q), np.asarray(W_k), np.asarray(w_v))
    queries = np.asarray(queries, np.float32)
    keys = np.asarray(keys, np.float32)
    values = np.asarray(values, np.float32)

    in_maps = []
    for c in range(NCORES):
        b, qh = c // 2, c % 2
        qTa = np.ascontiguousarray(
            queries[b, qh * NQH:(qh + 1) * NQH, :].T).astype(BF)
        kTa = np.ascontiguousarray(keys[b].T).astype(BF)
        # v4[p, ct, :] = values[b][ct*128 + p, :]
        v4a = np.ascontiguousarray(
            values[b].reshape(8, 128, 128).transpose(1, 0, 2)).astype(BF)
        in_maps.append({"qT": qTa, "kT": kTa, "v4": v4a,
                        "wqk": wqk, "cvec": cvec})

    kwargs = {}
    if _trace:
        kwargs["trace"] = True
        kwargs.update(_trace_kwargs or {})
    res = run_bass_kernel_spmd(nc, in_maps, core_ids=list(range(NCORES)), **kwargs)

    out = np.empty((BS, NQ, VD), np.float32)
    for c in range(NCORES):
        b, qh = c // 2, c % 2
        oT = np.asarray(res.results[c]["outT"]).astype(np.float32)  # [vd, q]
        s = np.asarray(res.results[c]["sums"])                       # [1, q]
        out[b, qh * NQH:(qh + 1) * NQH, :] = (oT / s).T
    if _trace:
        return out, res
    return out


# revision 17
# speedup vs baseline: 1.0242x; 1.0242x over previous
"""Additive (Bahdanau) attention on 8 TRN2 NeuronCores — V4.

Reference computation:
    qp = queries @ W_q                  (bs, n_q, 64)
    kp = keys @ W_k                     (bs, n_k, 64)
    scores[b,q,k] = sum_h w_v[h] * tanh(qp[b,q,h] + kp[b,k,h])
    out = softmax(scores, -1) @ values

tanh(x) ~= sum_{j<4} c_j sin((2j+1) w0 x): weighted-LS fit under the
empirical arg distribution (~N(0, sqrt2), |arg| <= 8.8), w0=0.336.
Angle addition makes the scores separable into matmuls with contraction
2*64 per harmonic. The HW Sin is a polynomial valid only on |x| <~ pi
(measured: err 6e-8 below 3, diverges after), so j>=1 args are range-
reduced exactly as V2: z = x/2pi + 36.xx (turns; packing phase in the
per-partition fraction), fp32 bit surgery keeps frac(z) (low 18 mantissa
bits, OR exponent 127 -> v in [1, 1.03125]), then Sin(-64pi v + 65pi)
lands in (-pi, pi].

Vs V2 (43.6us): J=4 refit (better e2e error than V2's J=5 minimax fit),
host-side transposed bf16 inputs feed the projections directly (no PE
transposes/casts/identity), v pre-tiled bf16, output bf16. Input DMAs
split across the idle sync/vector queues (a single DMA engine moves only
~90GB/s; parallel queues parallelize engines). j=0,1 banks bf16, (2,3)
packed fp8e4 DoubleRow. Exp on ScalarE; qp2 PSUM->SBUF copy on ScalarE
(DVE is the trig-phase bottleneck).

Sharding: fully data-parallel, no collectives. Core c handles batch c//2,
query half c%2: (512 q, 1024 k).
"""

import numpy as np

BS, NQ, NK = 4, 1024, 1024
QD, KD, VD, HID = 128, 128, 128, 64
NCORES = 8
NQH = NQ // 2  # queries per core

J = 4
W0 = 0.3360
FOURIER_C = [1.2083691, 0.29709556, 0.08487929, 0.04759338]
HALF_PI = 1.5707963267948966
TWO_PI = 6.283185307179586
PI64 = 64 * 3.141592653589793

N_DVE_EXP = 2  # tiles of exp computed on DVE (Schraudolph); rest ScalarE

_CACHED = {}


def _build():
    import concourse.bacc as bacc
    import concourse.mybir as mybir
    from concourse import tile
    from concourse.alu_op_type import AluOpType

    F32 = mybir.dt.float32
    U32 = mybir.dt.uint32
    BF16 = mybir.dt.bfloat16
    FP8 = mybir.dt.float8e4
    A = mybir.ActivationFunctionType
    DR = mybir.MatmulPerfMode.DoubleRow

    nc = bacc.Bacc(None, target_bir_lowering=False)

    qT = nc.declare_dram_parameter("qT", [QD, NQH], BF16, isOutput=False)
    kT = nc.declare_dram_parameter("kT", [KD, NK], BF16, isOutput=False)
    v4 = nc.declare_dram_parameter("v4", [128, 8, 128], BF16, isOutput=False)
    wqk = nc.declare_dram_parameter("wqk", [128, 256], BF16, isOutput=False)
    cvec = nc.declare_dram_parameter("cvec", [128, 16], F32, isOutput=False)
    outT = nc.declare_dram_parameter("outT", [128, NQH], BF16, isOutput=True)
    sums = nc.declare_dram_parameter("sums", [1, NQH], F32, isOutput=True)

    NQC = NQH // 128  # 4 query chunks
    NKC = NK // 128   # 8 key chunks
    S1 = [float((2 * j + 1) * W0 / TWO_PI) for j in range(J)]

    with tile.TileContext(nc) as tc:
        with (
            tc.tile_pool(name="consts", bufs=1) as consts,
            tc.tile_pool(name="io", bufs=1) as io,
            tc.tile_pool(name="work", bufs=4) as work,
            tc.tile_pool(name="jb", bufs=3) as jb,
            tc.tile_pool(name="sm", bufs=NKC) as sm,
            tc.tile_pool(name="ps", bufs=8, space="PSUM") as ps,
        ):
            # ---- input DMAs. k gates everything: its halves go first on
            # sync and scalar (descgen overlaps the scalar
            # table loads) so two DMA engines run in parallel; q follows
            # on sync. v/wqk/cvec on gpsimd.
            kT_sb = io.tile([KD, NK], BF16, tag="kT")
            qT_sb = io.tile([QD, NQH], BF16, tag="qT")
            wqk_sb = consts.tile([128, 256], BF16, tag="wqk")
            cvec_sb = consts.tile([128, 16], F32, tag="cvec")
            nc.sync.dma_start(kT_sb[:, 0:256], kT[:, 0:256])
            nc.scalar.dma_start(kT_sb[:, 256:512], kT[:, 256:512])
            nc.sync.dma_start(kT_sb[:, 512:768], kT[:, 512:768])
            nc.scalar.dma_start(kT_sb[:, 768:1024], kT[:, 768:1024])
            nc.gpsimd.dma_start(wqk_sb[:], wqk[:, :])
            nc.gpsimd.dma_start(cvec_sb[:], cvec[:, :])
            nc.sync.dma_start(qT_sb[:, 0:256], qT[:, 0:256])
            nc.gpsimd.dma_start(qT_sb[:, 256:512], qT[:, 256:512])
            sphq = cvec_sb[:, 0:1]
            sphk = cvec_sb[:, 1:2]
            biasq = cvec_sb[:, 2:3]
            biask = cvec_sb[:, 3:4]
            bias65 = cvec_sb[:, 4:5]

            ones16 = consts.tile([128, 1], BF16, tag="ones16")
            nc.gpsimd.memset(ones16[:], 1.0)



            # ---- projections (bf16, PE) ----
            wq_w = wqk_sb[:, 0:128]
            wk_w = wqk_sb[:, 128:256]
            pk0 = ps.tile([128, 512], F32, tag="t512", name="pk0")
            pk1 = ps.tile([128, 512], F32, tag="t512", name="pk1")
            pq = ps.tile([128, 512], F32, tag="t512", name="pq")
            nc.tensor.matmul(pk0[:], wk_w, kT_sb[:, 0:512], start=True, stop=True)
            nc.tensor.matmul(pk1[:], wk_w, kT_sb[:, 512:1024], start=True, stop=True)
            nc.tensor.matmul(pq[:], wq_w, qT_sb[:], start=True, stop=True)

            # kp to SBUF for the j>=1 preps: halves on DVE; qp2 on ScalarE
            # (keeps the DVE trig chain shorter).
            kp2 = io.tile([128, NK], F32, tag="kp2")
            qp2 = io.tile([128, NQH], F32, tag="qp2")
            nc.vector.tensor_copy(kp2[:, 0:512], pk0[:])
            nc.vector.tensor_copy(kp2[:, 512:1024], pk1[:])
            nc.vector.tensor_copy(qp2[:], pq[:])

            # ---- trig banks ----
            # K rows [cos_j | sin_j] unscaled; Q rows [sin_j | cos_j] * c_j w_v.
            # j=0,1 bf16; (2,3) fp8e4 packed for one DoubleRow pass.
            ksb = {j: jb.tile([128, NK], BF16, tag="ks", name=f"ks{j}")
                   for j in (0, 1)}
            sqb = {j: jb.tile([128, NQH], BF16, tag="sq", name=f"sq{j}")
                   for j in (0, 1)}
            kspair = jb.tile([128, 2, NK], FP8, tag="kspair")
            sqpair = jb.tile([128, 2, NQH], FP8, tag="sqpair")
            sqf = {j: work.tile([128, NQH], BF16, tag="sqf", name=f"sqf{j}",
                                bufs=4) for j in range(J)}

            def ks_dst(j):
                return kspair[:, j - 2, :] if j >= 2 else ksb[j][:]

            def sq_dst(j):
                return sqpair[:, j - 2, :] if j >= 2 else sqb[j][:]

            def scale_q(j):
                nc.vector.tensor_scalar_mul(sq_dst(j), sqf[j][:],
                                            cvec_sb[:, 5 + j:6 + j])

            # j=0 sins straight off the projection PSUMs.
            nc.scalar.activation(ksb[0][:, 0:512], pk0[:], A.Sin,
                                 bias=biask, scale=W0)
            nc.scalar.activation(ksb[0][:, 512:1024], pk1[:], A.Sin,
                                 bias=biask, scale=W0)
            nc.scalar.activation(sqf[0][:], pq[:], A.Sin,
                                 bias=biasq, scale=W0)
            scale_q(0)

            # j>=1: range-reduce on DVE (z in [32,64) turns incl. packing
            # phase; frac via bit surgery), Sin on ScalarE. Interleave the
            # q scales behind the matching sins.
            zk = {j: work.tile([128, NK], F32, tag="zk", name=f"zk{j}",
                               bufs=2) for j in (1, 2, 3)}
            vk = {j: work.tile([128, NK], F32, tag="vk", name=f"vk{j}",
                               bufs=3) for j in (1, 2, 3)}
            zq = {j: work.tile([128, NQH], F32, tag="zq", name=f"zq{j}",
                               bufs=2) for j in (1, 2, 3)}
            vq = {j: work.tile([128, NQH], F32, tag="vq", name=f"vq{j}",
                               bufs=3) for j in (1, 2, 3)}

            vq23 = work.tile([128, 2, NQH], F32, tag="vq23")
            vk23 = work.tile([128, 2, NK], F32, tag="vk23")
            sqf23 = work.tile([128, 2, NQH], BF16, tag="sqf23")

            def prep_q(j, dst):
                nc.vector.tensor_scalar(zq[j][:], qp2[:], S1[j], sphq,
                                        AluOpType.mult, AluOpType.add)
                nc.vector.tensor_scalar(dst.bitcast(U32),
                                        zq[j][:].bitcast(U32),
                                        0x0003FFFF, 0x3F800000,
                                        AluOpType.bitwise_and,
                                        AluOpType.bitwise_or)

            def prep_k(j, dst):
                nc.vector.tensor_scalar(zk[j][:], kp2[:], S1[j], sphk,
                                        AluOpType.mult, AluOpType.add)
                nc.vector.tensor_scalar(dst.bitcast(U32),
                                        zk[j][:].bitcast(U32),
                                        0x0003FFFF, 0x3F800000,
                                        AluOpType.bitwise_and,
                                        AluOpType.bitwise_or)

            # j=1 (bf16 banks)
            prep_q(1, vq[1][:])
            nc.scalar.activation(sqf[1][:], vq[1][:], A.Sin,
                                 scale=-PI64, bias=bias65)
            scale_q(1)
            prep_k(1, vk[1][:])
            nc.scalar.activation(ksb[1][:, 0:512], vk[1][:, 0:512], A.Sin,
                                 scale=-PI64, bias=bias65)
            nc.scalar.activation(ksb[1][:, 512:1024], vk[1][:, 512:1024],
                                 A.Sin, scale=-PI64, bias=bias65)
            v_sb = io.tile([128, 8, 128], BF16, tag="v4")
            nc.gpsimd.dma_start(v_sb[:], v4[:, :, :])
            # j=2,3: preps into pair tiles, one merged Sin per side feeding
            # the fp8 DoubleRow banks directly
            prep_q(2, vq23[:, 0, :])
            prep_q(3, vq23[:, 1, :])
            nc.scalar.activation(sqf23[:], vq23[:], A.Sin,
                                 scale=-PI64, bias=bias65)
            nc.vector.tensor_scalar_mul(sqpair[:, 0, :], sqf23[:, 0, :],
                                        cvec_sb[:, 7:8])
            nc.vector.tensor_scalar_mul(sqpair[:, 1, :], sqf23[:, 1, :],
                                        cvec_sb[:, 8:9])
            prep_k(2, vk23[:, 0, :])
            prep_k(3, vk23[:, 1, :])
            nc.scalar.activation(kspair[:], vk23[:], A.Sin,
                                 scale=-PI64, bias=bias65)

            # ---- score matmuls: psT[kt] = scores^T accumulated over j ----
            psT = [ps.tile([128, 512], F32, tag="t512", name=f"psT_{kt}")
                   for kt in range(NKC)]
            # p-state warmers: garbage matmuls into not-yet-started psT
            # banks (g0's start=True overwrites). The PE downclocks to
            # 1.2GHz when idle; these bridge the projections->g0 gap so
            # the score matmuls run at 2.4GHz.
            for w in range(6):
                nc.tensor.matmul(psT[w % 2][:], wqk_sb[:, 0:128],
                                 kT_sb[:, 0:512], start=True, stop=True)
            for kt in range(NKC):
                nc.tensor.matmul(psT[kt][:], ksb[0][:, kt * 128:(kt + 1) * 128],
                                 sqb[0][:], start=True, stop=False)
            for kt in range(NKC):
                nc.tensor.matmul(psT[kt][:], ksb[1][:, kt * 128:(kt + 1) * 128],
                                 sqb[1][:], start=False, stop=False)
            for kt in range(NKC // 2):
                nc.tensor.matmul(psT[kt][:],
                                 kspair[:, :, kt * 128:(kt + 1) * 128],
                                 sqpair[:], start=False, stop=True,
                                 perf_mode=DR)
            for kt in range(NKC // 2, NKC):
                nc.tensor.matmul(psT[kt][:],
                                 kspair[:, :, kt * 128:(kt + 1) * 128],
                                 sqpair[:], start=False, stop=True,
                                 perf_mode=DR)

            # ---- exp + denominators + output matmuls ----
            # First N_DVE_EXP tiles via the Schraudolph bf16 bit trick on
            # DVE (covers the ScalarE act-table reload); rest ScalarE Exp.
            I16 = mybir.dt.int16
            expT = []
            for kt in range(NKC):
                et = sm.tile([128, 512], BF16, tag="expT", name=f"expT_{kt}")
                if kt < N_DVE_EXP:
                    tf = work.tile([128, 512], F32, tag="schr", bufs=2,
                                   name=f"schr_{kt}")
                    # bits = s*2^7/ln2 + (127 - c)*2^7  (c~0.0455 incl
                    # trunc-vs-round slack), then f32->i16 convert writes
                    # the bf16 pattern of e^s.
                    nc.vector.tensor_scalar(tf[:], psT[kt][:],
                                            184.6650, 16250.18,
                                            AluOpType.mult, AluOpType.add)
                    nc.vector.tensor_copy(et[:].bitcast(I16), tf[:])
                else:
                    nc.scalar.activation(et[:], psT[kt][:], A.Exp)
                expT.append(et)

            ps_outT = ps.tile([128, 512], F32, tag="t512", name="ps_outT")
            psum_sums = ps.tile([1, 512], F32, tag="t512", name="psum_sums")
            for kt in range(NKC):
                nc.tensor.matmul(ps_outT[:], v_sb[:, kt, :], expT[kt][:],
                                 start=(kt == 0), stop=(kt == NKC - 1))
                nc.tensor.matmul(psum_sums[:], ones16[:], expT[kt][:],
                                 start=(kt == 0), stop=(kt == NKC - 1))
            sums_sb = sm.tile([1, 512], F32, tag="sums_sb")
            nc.scalar.activation(sums_sb[:], psum_sums[:], A.Copy)
            nc.gpsimd.dma_start(sums[:, :], sums_sb[:])
            outT_sb = sm.tile([128, 512], BF16, tag="outT_sb")
            nc.vector.tensor_copy(outT_sb[:, 0:256], ps_outT[:, 0:256])
            nc.sync.dma_start(outT[:, 0:256], outT_sb[:, 0:256])
            nc.vector.tensor_copy(outT_sb[:, 256:512], ps_outT[:, 256:512])
            nc.scalar.dma_start(outT[:, 256:512], outT_sb[:, 256:512])

    nc.finalize()
    return nc


def _get_nc():
    if "nc" not in _CACHED:
        _CACHED["nc"] = _build()
    return _CACHED["nc"]


def _make_consts(W_q, W_k, w_v):
    import ml_dtypes
    # wqk layout: [:, 0:128] = [W_q | W_q], [:, 128:256] = [W_k | W_k]
    wqk = np.zeros((128, 256), np.float32)
    wqk[:, 0:64] = W_q
    wqk[:, 64:128] = W_q
    wqk[:, 128:192] = W_k
    wqk[:, 192:256] = W_k
    cvec = np.zeros((128, 16), np.float32)
    # wrap-phase consts (turns, +36 so z lands in [32, 64)):
    # Q packing [sin | cos], K packing [cos | sin]
    cvec[:64, 0] = 36.0
    cvec[64:, 0] = 36.25   # sphq
    cvec[:64, 1] = 36.25
    cvec[64:, 1] = 36.0    # sphk
    cvec[64:, 2] = HALF_PI  # biasq (radians, j=0 direct)
    cvec[:64, 3] = HALF_PI  # biask
    cvec[:, 4] = 65 * np.pi  # bias65
    for j in range(J):
        cwj = (FOURIER_C[j] * w_v).astype(np.float32)
        cvec[:64, 5 + j] = cwj
        cvec[64:, 5 + j] = cwj
    return wqk.astype(ml_dtypes.bfloat16), cvec


def kernel(queries, keys, values, W_q, W_k, w_v, _trace=False, _trace_kwargs=None):
    import ml_dtypes
    from concourse.bass_utils import run_bass_kernel_spmd

    BF = ml_dtypes.bfloat16
    nc = _get_nc()
    wqk, cvec = _make_consts(
        np.asarray(W_q), np.asarray(W_k), np.asarray(w_v))
    queries = np.asarray(queries, np.float32)
    keys = np.asarray(keys, np.float32)
    values = np.asarray(values, np.float32)

    in_maps = []
    for c in range(NCORES):
        b, qh = c // 2, c % 2
        qTa = np.ascontiguousarray(
            queries[b, qh * NQH:(qh + 1) * NQH, :].T).astype(BF)
        kTa = np.ascontiguousarray(keys[b].T).astype(BF)
        # v4[p, ct, :] = values[b][ct*128 + p, :]
        v4a = np.ascontiguousarray(
            values[b].reshape(8, 128, 128).transpose(1, 0, 2)).astype(BF)
        in_maps.append({"qT": qTa, "kT": kTa, "v4": v4a,
                        "wqk": wqk, "cvec": cvec})

    kwargs = {}
    if _trace:
        kwargs["trace"] = True
        kwargs.update(_trace_kwargs or {})
    res = run_bass_kernel_spmd(nc, in_maps, core_ids=list(range(NCORES)), **kwargs)

    out = np.empty((BS, NQ, VD), np.float32)
    for c in range(NCORES):
        b, qh = c // 2, c % 2
        oT = np.asarray(res.results[c]["outT"]).astype(np.float32)  # [vd, q]
        s = np.asarray(res.results[c]["sums"])                       # [1, q]
        out[b, qh * NQH:(qh + 1) * NQH, :] = (oT / s).T
    if _trace:
        return out, res
    return out


# revision 18
# speedup vs baseline: 1.0387x; 1.0141x over previous
"""Additive (Bahdanau) attention on 8 TRN2 NeuronCores — V4.

Reference computation:
    qp = queries @ W_q                  (bs, n_q, 64)
    kp = keys @ W_k                     (bs, n_k, 64)
    scores[b,q,k] = sum_h w_v[h] * tanh(qp[b,q,h] + kp[b,k,h])
    out = softmax(scores, -1) @ values

tanh(x) ~= sum_{j<4} c_j sin((2j+1) w0 x): weighted-LS fit under the
empirical arg distribution (~N(0, sqrt2), |arg| <= 8.8), w0=0.336.
Angle addition makes the scores separable into matmuls with contraction
2*64 per harmonic. The HW Sin is a polynomial valid only on |x| <~ pi
(measured: err 6e-8 below 3, diverges after), so j>=1 args are range-
reduced exactly as V2: z = x/2pi + 36.xx (turns; packing phase in the
per-partition fraction), fp32 bit surgery keeps frac(z) (low 18 mantissa
bits, OR exponent 127 -> v in [1, 1.03125]), then Sin(-64pi v + 65pi)
lands in (-pi, pi].

Vs V2 (43.6us): J=4 refit (better e2e error than V2's J=5 minimax fit),
host-side transposed bf16 inputs feed the projections directly (no PE
transposes/casts/identity), v pre-tiled bf16, output bf16. Input DMAs
split across the idle sync/vector queues (a single DMA engine moves only
~90GB/s; parallel queues parallelize engines). j=0,1 banks bf16, (2,3)
packed fp8e4 DoubleRow. Exp on ScalarE; qp2 PSUM->SBUF copy on ScalarE
(DVE is the trig-phase bottleneck).

Sharding: fully data-parallel, no collectives. Core c handles batch c//2,
query half c%2: (512 q, 1024 k).
"""

import numpy as np

BS, NQ, NK = 4, 1024, 1024
QD, KD, VD, HID = 128, 128, 128, 64
NCORES = 8
NQH = NQ // 2  # queries per core

J = 4
W0 = 0.3360
FOURIER_C = [1.2083691, 0.29709556, 0.08487929, 0.04759338]
HALF_PI = 1.5707963267948966
TWO_PI = 6.283185307179586
PI64 = 64 * 3.141592653589793

N_DVE_EXP = 2  # tiles of exp computed on DVE (Schraudolph); rest ScalarE

_CACHED = {}


def _build():
    import concourse.bacc as bacc
    import concourse.mybir as mybir
    from concourse import tile
    from concourse.alu_op_type import AluOpType

    F32 = mybir.dt.float32
    U32 = mybir.dt.uint32
    BF16 = mybir.dt.bfloat16
    FP8 = mybir.dt.float8e4
    A = mybir.ActivationFunctionType
    DR = mybir.MatmulPerfMode.DoubleRow

    nc = bacc.Bacc(None, target_bir_lowering=False)

    qT = nc.declare_dram_parameter("qT", [QD, NQH], BF16, isOutput=False)
    kT = nc.declare_dram_parameter("kT", [KD, NK], BF16, isOutput=False)
    v4 = nc.declare_dram_parameter("v4", [128, 8, 128], BF16, isOutput=False)
    wqk = nc.declare_dram_parameter("wqk", [128, 256], BF16, isOutput=False)
    cvec = nc.declare_dram_parameter("cvec", [128, 16], F32, isOutput=False)
    outT = nc.declare_dram_parameter("outT", [128, NQH], BF16, isOutput=True)
    vdly = nc.declare_dram_parameter("vdly", [1, 1], BF16, isOutput=True)
    sums = nc.declare_dram_parameter("sums", [1, NQH], F32, isOutput=True)

    NQC = NQH // 128  # 4 query chunks
    NKC = NK // 128   # 8 key chunks
    S1 = [float((2 * j + 1) * W0 / TWO_PI) for j in range(J)]

    with tile.TileContext(nc) as tc:
        with (
            tc.tile_pool(name="consts", bufs=1) as consts,
            tc.tile_pool(name="io", bufs=1) as io,
            tc.tile_pool(name="work", bufs=4) as work,
            tc.tile_pool(name="jb", bufs=3) as jb,
            tc.tile_pool(name="sm", bufs=NKC) as sm,
            tc.tile_pool(name="ps", bufs=8, space="PSUM") as ps,
        ):
            # ---- input DMAs. k gates everything: its halves go first on
            # sync and scalar (descgen overlaps the scalar
            # table loads) so two DMA engines run in parallel; q follows
            # on sync. v/wqk/cvec on gpsimd.
            kT_sb = io.tile([KD, NK], BF16, tag="kT")
            qT_sb = io.tile([QD, NQH], BF16, tag="qT")
            wqk_sb = consts.tile([128, 256], BF16, tag="wqk")
            cvec_sb = consts.tile([128, 16], F32, tag="cvec")
            nc.sync.dma_start(kT_sb[:, 0:256], kT[:, 0:256])
            nc.scalar.dma_start(kT_sb[:, 256:512], kT[:, 256:512])
            nc.sync.dma_start(kT_sb[:, 512:768], kT[:, 512:768])
            nc.scalar.dma_start(kT_sb[:, 768:1024], kT[:, 768:1024])
            nc.gpsimd.dma_start(wqk_sb[:], wqk[:, :])
            nc.gpsimd.dma_start(cvec_sb[:], cvec[:, :])
            nc.sync.dma_start(qT_sb[:, 0:256], qT[:, 0:256])
            nc.gpsimd.dma_start(qT_sb[:, 256:512], qT[:, 256:512])
            sphq = cvec_sb[:, 0:1]
            sphk = cvec_sb[:, 1:2]
            biasq = cvec_sb[:, 2:3]
            biask = cvec_sb[:, 3:4]
            bias65 = cvec_sb[:, 4:5]

            ones16 = consts.tile([128, 1], BF16, tag="ones16")
            nc.gpsimd.memset(ones16[:], 1.0)



            # ---- projections (bf16, PE) ----
            wq_w = wqk_sb[:, 0:128]
            wk_w = wqk_sb[:, 128:256]
            pk0 = ps.tile([128, 512], F32, tag="t512", name="pk0")
            pk1 = ps.tile([128, 512], F32, tag="t512", name="pk1")
            pq = ps.tile([128, 512], F32, tag="t512", name="pq")
            nc.tensor.matmul(pk0[:], wk_w, kT_sb[:, 0:512], start=True, stop=True)
            nc.tensor.matmul(pk1[:], wk_w, kT_sb[:, 512:1024], start=True, stop=True)
            nc.tensor.matmul(pq[:], wq_w, qT_sb[:], start=True, stop=True)

            # kp to SBUF for the j>=1 preps: halves on DVE; qp2 on ScalarE
            # (keeps the DVE trig chain shorter).
            kp2 = io.tile([128, NK], F32, tag="kp2")
            qp2 = io.tile([128, NQH], F32, tag="qp2")
            nc.vector.tensor_copy(kp2[:, 0:512], pk0[:])
            nc.vector.tensor_copy(kp2[:, 512:1024], pk1[:])
            nc.vector.tensor_copy(qp2[:], pq[:])

            # ---- trig banks ----
            # K rows [cos_j | sin_j] unscaled; Q rows [sin_j | cos_j] * c_j w_v.
            # j=0,1 bf16; (2,3) fp8e4 packed for one DoubleRow pass.
            ksb = {j: jb.tile([128, NK], BF16, tag="ks", name=f"ks{j}")
                   for j in (0, 1)}
            sqb = {j: jb.tile([128, NQH], BF16, tag="sq", name=f"sq{j}")
                   for j in (0, 1)}
            kspair = jb.tile([128, 2, NK], FP8, tag="kspair")
            sqpair = jb.tile([128, 2, NQH], FP8, tag="sqpair")
            sqf = {j: work.tile([128, NQH], BF16, tag="sqf", name=f"sqf{j}",
                                bufs=4) for j in range(J)}

            def ks_dst(j):
                return kspair[:, j - 2, :] if j >= 2 else ksb[j][:]

            def sq_dst(j):
                return sqpair[:, j - 2, :] if j >= 2 else sqb[j][:]

            def scale_q(j):
                nc.vector.tensor_scalar_mul(sq_dst(j), sqf[j][:],
                                            cvec_sb[:, 5 + j:6 + j])

            # j=0 sins straight off the projection PSUMs.
            nc.scalar.activation(ksb[0][:, 0:512], pk0[:], A.Sin,
                                 bias=biask, scale=W0)
            nc.scalar.activation(ksb[0][:, 512:1024], pk1[:], A.Sin,
                                 bias=biask, scale=W0)
            nc.scalar.activation(sqf[0][:], pq[:], A.Sin,
                                 bias=biasq, scale=W0)
            scale_q(0)

            # j>=1: range-reduce on DVE (z in [32,64) turns incl. packing
            # phase; frac via bit surgery), Sin on ScalarE. Interleave the
            # q scales behind the matching sins.
            zk = {j: work.tile([128, NK], F32, tag="zk", name=f"zk{j}",
                               bufs=2) for j in (1, 2, 3)}
            vk = {j: work.tile([128, NK], F32, tag="vk", name=f"vk{j}",
                               bufs=3) for j in (1, 2, 3)}
            zq = {j: work.tile([128, NQH], F32, tag="zq", name=f"zq{j}",
                               bufs=2) for j in (1, 2, 3)}
            vq = {j: work.tile([128, NQH], F32, tag="vq", name=f"vq{j}",
                               bufs=3) for j in (1, 2, 3)}

            vq23 = work.tile([128, 2, NQH], F32, tag="vq23")
            vk23 = work.tile([128, 2, NK], F32, tag="vk23")
            zq23 = work.tile([128, 2, NQH], F32, tag="zq23")
            zk23 = work.tile([128, 2, NK], F32, tag="zk23")
            sqf23 = work.tile([128, 2, NQH], BF16, tag="sqf23")

            # j=1 (bf16 banks)
            nc.vector.tensor_scalar(zq[1][:], qp2[:], S1[1], sphq,
                                    AluOpType.mult, AluOpType.add)
            nc.vector.tensor_scalar(vq[1][:].bitcast(U32),
                                    zq[1][:].bitcast(U32),
                                    0x0003FFFF, 0x3F800000,
                                    AluOpType.bitwise_and,
                                    AluOpType.bitwise_or)
            nc.scalar.activation(sqf[1][:], vq[1][:], A.Sin,
                                 scale=-PI64, bias=bias65)
            scale_q(1)
            nc.vector.tensor_scalar(zk[1][:], kp2[:], S1[1], sphk,
                                    AluOpType.mult, AluOpType.add)
            nc.vector.tensor_scalar(vk[1][:].bitcast(U32),
                                    zk[1][:].bitcast(U32),
                                    0x0003FFFF, 0x3F800000,
                                    AluOpType.bitwise_and,
                                    AluOpType.bitwise_or)
            nc.scalar.activation(ksb[1][:, 0:512], vk[1][:, 0:512], A.Sin,
                                 scale=-PI64, bias=bias65)
            nc.scalar.activation(ksb[1][:, 512:1024], vk[1][:, 512:1024],
                                 A.Sin, scale=-PI64, bias=bias65)
            # v4 input: gated behind a 4-byte store that depends on ksb[1],
            # so its transfer stays clear of the set9 table-load window
            v_sb = io.tile([128, 8, 128], BF16, tag="v4")
            nc.sync.dma_start(vdly[:, :], ksb[1][:1, 0:1])
            nc.sync.dma_start(v_sb[:], v4[:, :, :])
            # j=2,3: z into pair tiles, one bitwise + one Sin per side
            nc.vector.tensor_scalar(zq23[:, 0, :], qp2[:], S1[2], sphq,
                                    AluOpType.mult, AluOpType.add)
            nc.vector.tensor_scalar(zq23[:, 1, :], qp2[:], S1[3], sphq,
                                    AluOpType.mult, AluOpType.add)
            nc.vector.tensor_scalar(vq23[:].bitcast(U32),
                                    zq23[:].bitcast(U32),
                                    0x0003FFFF, 0x3F800000,
                                    AluOpType.bitwise_and,
                                    AluOpType.bitwise_or)
            nc.scalar.activation(sqf23[:], vq23[:], A.Sin,
                                 scale=-PI64, bias=bias65)
            nc.vector.tensor_scalar_mul(sqpair[:, 0, :], sqf23[:, 0, :],
                                        cvec_sb[:, 7:8])
            nc.vector.tensor_scalar_mul(sqpair[:, 1, :], sqf23[:, 1, :],
                                        cvec_sb[:, 8:9])
            nc.vector.tensor_scalar(zk23[:, 0, :], kp2[:], S1[2], sphk,
                                    AluOpType.mult, AluOpType.add)
            nc.vector.tensor_scalar(zk23[:, 1, :], kp2[:], S1[3], sphk,
                                    AluOpType.mult, AluOpType.add)
            nc.vector.tensor_scalar(vk23[:].bitcast(U32),
                                    zk23[:].bitcast(U32),
                                    0x0003FFFF, 0x3F800000,
                                    AluOpType.bitwise_and,
                                    AluOpType.bitwise_or)
            nc.scalar.activation(kspair[:], vk23[:], A.Sin,
                                 scale=-PI64, bias=bias65)

            # ---- score matmuls: psT[kt] = scores^T accumulated over j ----
            psT = [ps.tile([128, 512], F32, tag="t512", name=f"psT_{kt}")
                   for kt in range(NKC)]
            # p-state warmers: garbage matmuls into not-yet-started psT
            # banks (g0's start=True overwrites). The PE downclocks to
            # 1.2GHz when idle; these bridge the projections->g0 gap so
            # the score matmuls run at 2.4GHz.
            for w in range(6):
                nc.tensor.matmul(psT[w % 2][:], wqk_sb[:, 0:128],
                                 kT_sb[:, 0:512], start=True, stop=True)
            for kt in range(NKC):
                nc.tensor.matmul(psT[kt][:], ksb[0][:, kt * 128:(kt + 1) * 128],
                                 sqb[0][:], start=True, stop=False)
            for kt in range(NKC):
                nc.tensor.matmul(psT[kt][:], ksb[1][:, kt * 128:(kt + 1) * 128],
                                 sqb[1][:], start=False, stop=False)
            for kt in range(NKC // 2):
                nc.tensor.matmul(psT[kt][:],
                                 kspair[:, :, kt * 128:(kt + 1) * 128],
                                 sqpair[:], start=False, stop=True,
                                 perf_mode=DR)
            for kt in range(NKC // 2, NKC):
                nc.tensor.matmul(psT[kt][:],
                                 kspair[:, :, kt * 128:(kt + 1) * 128],
                                 sqpair[:], start=False, stop=True,
                                 perf_mode=DR)

            # ---- exp + denominators + output matmuls ----
            # First N_DVE_EXP tiles via the Schraudolph bf16 bit trick on
            # DVE (covers the ScalarE act-table reload); rest ScalarE Exp.
            I16 = mybir.dt.int16
            expT = []
            for kt in range(NKC):
                et = sm.tile([128, 512], BF16, tag="expT", name=f"expT_{kt}")
                if kt < N_DVE_EXP:
                    tf = work.tile([128, 512], F32, tag="schr", bufs=2,
                                   name=f"schr_{kt}")
                    # bits = s*2^7/ln2 + (127 - c)*2^7  (c~0.0455 incl
                    # trunc-vs-round slack), then f32->i16 convert writes
                    # the bf16 pattern of e^s.
                    nc.vector.tensor_scalar(tf[:], psT[kt][:],
                                            184.6650, 16250.18,
                                            AluOpType.mult, AluOpType.add)
                    nc.vector.tensor_copy(et[:].bitcast(I16), tf[:])
                else:
                    nc.scalar.activation(et[:], psT[kt][:], A.Exp)
                expT.append(et)

            ps_outT = ps.tile([128, 512], F32, tag="t512", name="ps_outT")
            psum_sums = ps.tile([1, 512], F32, tag="t512", name="psum_sums")
            for kt in range(NKC):
                nc.tensor.matmul(ps_outT[:], v_sb[:, kt, :], expT[kt][:],
                                 start=(kt == 0), stop=(kt == NKC - 1))
                nc.tensor.matmul(psum_sums[:], ones16[:], expT[kt][:],
                                 start=(kt == 0), stop=(kt == NKC - 1))
            sums_sb = sm.tile([1, 512], F32, tag="sums_sb")
            nc.scalar.activation(sums_sb[:], psum_sums[:], A.Copy)
            nc.gpsimd.dma_start(sums[:, :], sums_sb[:])
            outT_sb = sm.tile([128, 512], BF16, tag="outT_sb")
            nc.vector.tensor_copy(outT_sb[:, 0:256], ps_outT[:, 0:256])
            nc.sync.dma_start(outT[:, 0:256], outT_sb[:, 0:256])
            nc.vector.tensor_copy(outT_sb[:, 256:512], ps_outT[:, 256:512])
            nc.scalar.dma_start(outT[:, 256:512], outT_sb[:, 256:512])

    nc.finalize()
    return nc


def _get_nc():
    if "nc" not in _CACHED:
        _CACHED["nc"] = _build()
    return _CACHED["nc"]


def _make_consts(W_q, W_k, w_v):
    import ml_dtypes
    # wqk layout: [:, 0:128] = [W_q | W_q], [:, 128:256] = [W_k | W_k]
    wqk = np.zeros((128, 256), np.float32)
    wqk[:, 0:64] = W_q
    wqk[:, 64:128] = W_q
    wqk[:, 128:192] = W_k
    wqk[:, 192:256] = W_k
    cvec = np.zeros((128, 16), np.float32)
    # wrap-phase consts (turns, +36 so z lands in [32, 64)):
    # Q packing [sin | cos], K packing [cos | sin]
    cvec[:64, 0] = 36.0
    cvec[64:, 0] = 36.25   # sphq
    cvec[:64, 1] = 36.25
    cvec[64:, 1] = 36.0    # sphk
    cvec[64:, 2] = HALF_PI  # biasq (radians, j=0 direct)
    cvec[:64, 3] = HALF_PI  # biask
    cvec[:, 4] = 65 * np.pi  # bias65
    for j in range(J):
        cwj = (FOURIER_C[j] * w_v).astype(np.float32)
        cvec[:64, 5 + j] = cwj
        cvec[64:, 5 + j] = cwj
    return wqk.astype(ml_dtypes.bfloat16), cvec


def kernel(queries, keys, values, W_q, W_k, w_v, _trace=False, _trace_kwargs=None):
    import ml_dtypes
    from concourse.bass_utils import run_bass_kernel_spmd

    BF = ml_dtypes.bfloat16
    nc = _get_nc()
    wqk, cvec = _make_consts(
        np.asarray(W_q), np.asarray(W_k), np.asarray(w_v))
    queries = np.asarray(queries, np.float32)
    keys = np.asarray(keys, np.float32)
    values = np.asarray(values, np.float32)

    in_maps = []
    for c in range(NCORES):
        b, qh = c // 2, c % 2
        qTa = np.ascontiguousarray(
            queries[b, qh * NQH:(qh + 1) * NQH, :].T).astype(BF)
        kTa = np.ascontiguousarray(keys[b].T).astype(BF)
        # v4[p, ct, :] = values[b][ct*128 + p, :]
        v4a = np.ascontiguousarray(
            values[b].reshape(8, 128, 128).transpose(1, 0, 2)).astype(BF)
        in_maps.append({"qT": qTa, "kT": kTa, "v4": v4a,
                        "wqk": wqk, "cvec": cvec})

    kwargs = {}
    if _trace:
        kwargs["trace"] = True
        kwargs.update(_trace_kwargs or {})
    res = run_bass_kernel_spmd(nc, in_maps, core_ids=list(range(NCORES)), **kwargs)

    out = np.empty((BS, NQ, VD), np.float32)
    for c in range(NCORES):
        b, qh = c // 2, c % 2
        oT = np.asarray(res.results[c]["outT"]).astype(np.float32)  # [vd, q]
        s = np.asarray(res.results[c]["sums"])                       # [1, q]
        out[b, qh * NQH:(qh + 1) * NQH, :] = (oT / s).T
    if _trace:
        return out, res
    return out


# revision 21
# speedup vs baseline: 1.0550x; 1.0157x over previous
"""Additive (Bahdanau) attention on 8 TRN2 NeuronCores — V4.

Reference computation:
    qp = queries @ W_q                  (bs, n_q, 64)
    kp = keys @ W_k                     (bs, n_k, 64)
    scores[b,q,k] = sum_h w_v[h] * tanh(qp[b,q,h] + kp[b,k,h])
    out = softmax(scores, -1) @ values

tanh(x) ~= sum_{j<4} c_j sin((2j+1) w0 x): weighted-LS fit under the
empirical arg distribution (~N(0, sqrt2), |arg| <= 8.8), w0=0.336.
Angle addition makes the scores separable into matmuls with contraction
2*64 per harmonic. The HW Sin is a polynomial valid only on |x| <~ pi
(measured: err 6e-8 below 3, diverges after), so j>=1 args are range-
reduced exactly as V2: z = x/2pi + 36.xx (turns; packing phase in the
per-partition fraction), fp32 bit surgery keeps frac(z) (low 18 mantissa
bits, OR exponent 127 -> v in [1, 1.03125]), then Sin(-64pi v + 65pi)
lands in (-pi, pi].

Vs V2 (43.6us): J=4 refit (better e2e error than V2's J=5 minimax fit),
host-side transposed bf16 inputs feed the projections directly (no PE
transposes/casts/identity), v pre-tiled bf16, output bf16. Input DMAs
split across the idle sync/vector queues (a single DMA engine moves only
~90GB/s; parallel queues parallelize engines). j=0,1 banks bf16, (2,3)
packed fp8e4 DoubleRow. Exp on ScalarE; qp2 PSUM->SBUF copy on ScalarE
(DVE is the trig-phase bottleneck).

Sharding: fully data-parallel, no collectives. Core c handles batch c//2,
query half c%2: (512 q, 1024 k).
"""

import numpy as np

BS, NQ, NK = 4, 1024, 1024
QD, KD, VD, HID = 128, 128, 128, 64
NCORES = 8
NQH = NQ // 2  # queries per core

J = 4
W0 = 0.3360
FOURIER_C = [1.2083691, 0.29709556, 0.08487929, 0.04759338]
HALF_PI = 1.5707963267948966
TWO_PI = 6.283185307179586
PI64 = 64 * 3.141592653589793

N_DVE_EXP = 0  # tiles of exp computed on DVE (Schraudolph); rest ScalarE

_CACHED = {}


def _build():
    import concourse.bacc as bacc
    import concourse.mybir as mybir
    from concourse import tile
    from concourse.alu_op_type import AluOpType

    F32 = mybir.dt.float32
    U32 = mybir.dt.uint32
    BF16 = mybir.dt.bfloat16
    FP8 = mybir.dt.float8e4
    A = mybir.ActivationFunctionType
    DR = mybir.MatmulPerfMode.DoubleRow

    nc = bacc.Bacc(None, target_bir_lowering=False)

    qT = nc.declare_dram_parameter("qT", [QD, NQH], BF16, isOutput=False)
    kT = nc.declare_dram_parameter("kT", [KD, NK], BF16, isOutput=False)
    v4 = nc.declare_dram_parameter("v4", [128, 8, 128], BF16, isOutput=False)
    wqk = nc.declare_dram_parameter("wqk", [128, 256], BF16, isOutput=False)
    cvec = nc.declare_dram_parameter("cvec", [128, 16], F32, isOutput=False)
    outT = nc.declare_dram_parameter("outT", [128, NQH], BF16, isOutput=True)
    sums = nc.declare_dram_parameter("sums", [1, NQH], F32, isOutput=True)

    NQC = NQH // 128  # 4 query chunks
    NKC = NK // 128   # 8 key chunks
    S1 = [float((2 * j + 1) * W0 / TWO_PI) for j in range(J)]

    with tile.TileContext(nc) as tc:
        with (
            tc.tile_pool(name="consts", bufs=1) as consts,
            tc.tile_pool(name="io", bufs=1) as io,
            tc.tile_pool(name="work", bufs=4) as work,
            tc.tile_pool(name="jb", bufs=3) as jb,
            tc.tile_pool(name="sm", bufs=NKC) as sm,
            tc.tile_pool(name="ps", bufs=8, space="PSUM") as ps,
        ):
            # ---- input DMAs. k gates everything: its halves go first on
            # sync and scalar (descgen overlaps the scalar
            # table loads) so two DMA engines run in parallel; q follows
            # on sync. v/wqk/cvec on gpsimd.
            kT_sb = io.tile([KD, NK], BF16, tag="kT")
            qT_sb = io.tile([QD, NQH], BF16, tag="qT")
            wqk_sb = consts.tile([128, 256], BF16, tag="wqk")
            cvec_sb = consts.tile([128, 16], F32, tag="cvec")
            nc.sync.dma_start(kT_sb[:, 0:256], kT[:, 0:256])
            nc.scalar.dma_start(kT_sb[:, 256:512], kT[:, 256:512])
            nc.sync.dma_start(kT_sb[:, 512:768], kT[:, 512:768])
            nc.scalar.dma_start(kT_sb[:, 768:1024], kT[:, 768:1024])
            nc.gpsimd.dma_start(wqk_sb[:], wqk[:, :])
            nc.gpsimd.dma_start(cvec_sb[:], cvec[:, :])
            nc.sync.dma_start(qT_sb[:, 0:256], qT[:, 0:256])
            nc.gpsimd.dma_start(qT_sb[:, 256:512], qT[:, 256:512])
            sphq = cvec_sb[:, 0:1]
            sphk = cvec_sb[:, 1:2]
            biasq = cvec_sb[:, 2:3]
            biask = cvec_sb[:, 3:4]
            bias65 = cvec_sb[:, 4:5]

            ones16 = consts.tile([128, 1], BF16, tag="ones16")
            nc.gpsimd.memset(ones16[:], 1.0)
            zeros16 = consts.tile([128, 128], BF16, tag="zeros16")
            nc.gpsimd.memset(zeros16[:], 0.0)


            v_sb = io.tile([128, 8, 128], BF16, tag="v4")

            # ---- projections (bf16, PE) ----
            wq_w = wqk_sb[:, 0:128]
            wk_w = wqk_sb[:, 128:256]
            pk0 = ps.tile([128, 512], F32, tag="t512", name="pk0")
            pk1 = ps.tile([128, 512], F32, tag="t512", name="pk1")
            pq = ps.tile([128, 512], F32, tag="t512", name="pq")
            nc.tensor.matmul(pk0[:], wk_w, kT_sb[:, 0:512], start=True, stop=True)
            nc.tensor.matmul(pk1[:], wk_w, kT_sb[:, 512:1024], start=True, stop=True)
            nc.tensor.matmul(pq[:], wq_w, qT_sb[:], start=True, stop=True)

            # kp to SBUF for the j>=1 preps: halves on DVE; qp2 on ScalarE
            # (keeps the DVE trig chain shorter).
            kp2 = io.tile([128, NK], F32, tag="kp2")
            qp2 = io.tile([128, NQH], F32, tag="qp2")
            nc.vector.tensor_copy(kp2[:, 0:512], pk0[:])
            nc.vector.tensor_copy(kp2[:, 512:1024], pk1[:])

            # ---- trig banks ----
            # K rows [cos_j | sin_j] unscaled; Q rows [sin_j | cos_j] * c_j w_v.
            # j=0,1 bf16; (2,3) fp8e4 packed for one DoubleRow pass.
            ksb = {j: jb.tile([128, NK], BF16, tag="ks", name=f"ks{j}")
                   for j in (0, 1)}
            sqb = {j: jb.tile([128, NQH], BF16, tag="sq", name=f"sq{j}")
                   for j in (0, 1)}
            kspair = jb.tile([128, 2, NK], FP8, tag="kspair")
            sqpair = jb.tile([128, 2, NQH], FP8, tag="sqpair")
            sqf = {j: work.tile([128, NQH], BF16, tag="sqf", name=f"sqf{j}",
                                bufs=4) for j in range(J)}

            def ks_dst(j):
                return kspair[:, j - 2, :] if j >= 2 else ksb[j][:]

            def sq_dst(j):
                return sqpair[:, j - 2, :] if j >= 2 else sqb[j][:]

            def scale_q(j):
                nc.vector.tensor_scalar_mul(sq_dst(j), sqf[j][:],
                                            cvec_sb[:, 5 + j:6 + j])

            # j=0 sins straight off the projection PSUMs.
            nc.scalar.activation(ksb[0][:, 0:512], pk0[:], A.Sin,
                                 bias=biask, scale=W0)
            nc.scalar.activation(ksb[0][:, 512:1024], pk1[:], A.Sin,
                                 bias=biask, scale=W0)
            nc.scalar.activation(sqf[0][:], pq[:], A.Sin,
                                 bias=biasq, scale=W0)

            # j>=1: range-reduce on DVE (z in [32,64) turns incl. packing
            # phase; frac via bit surgery), Sin on ScalarE. Interleave the
            # q scales behind the matching sins.
            zk = {j: work.tile([128, NK], F32, tag="zk", name=f"zk{j}",
                               bufs=2) for j in (1, 2, 3)}
            vk = {j: work.tile([128, NK], F32, tag="vk", name=f"vk{j}",
                               bufs=3) for j in (1, 2, 3)}
            zq = {j: work.tile([128, NQH], F32, tag="zq", name=f"zq{j}",
                               bufs=2) for j in (1, 2, 3)}
            vq = {j: work.tile([128, NQH], F32, tag="vq", name=f"vq{j}",
                               bufs=3) for j in (1, 2, 3)}

            for j in (1, 2, 3):
                nc.vector.tensor_scalar(zk[j][:], kp2[:], S1[j], sphk,
                                        AluOpType.mult, AluOpType.add)
                nc.vector.tensor_scalar(vk[j][:].bitcast(U32),
                                        zk[j][:].bitcast(U32),
                                        0x0003FFFF, 0x3F800000,
                                        AluOpType.bitwise_and,
                                        AluOpType.bitwise_or)
                nc.scalar.activation(ks_dst(j), vk[j][:], A.Sin,
                                     scale=-PI64, bias=bias65)
                if j == 1:
                    nc.scalar.activation(qp2[:], pq[:], A.Copy)
                    nc.gpsimd.dma_start(v_sb[:], v4[:, :, :])
                nc.vector.tensor_scalar(zq[j][:], qp2[:], S1[j], sphq,
                                        AluOpType.mult, AluOpType.add)
                nc.vector.tensor_scalar(vq[j][:].bitcast(U32),
                                        zq[j][:].bitcast(U32),
                                        0x0003FFFF, 0x3F800000,
                                        AluOpType.bitwise_and,
                                        AluOpType.bitwise_or)
                nc.scalar.activation(sqf[j][:], vq[j][:], A.Sin,
                                     scale=-PI64, bias=bias65)
                scale_q(j - 1)
            scale_q(3)

            # ---- score matmuls: psT[kt] = scores^T accumulated over j ----
            psT = [ps.tile([128, 512], F32, tag="t512", name=f"psT_{kt}")
                   for kt in range(NKC)]
            # p-state warmers: garbage matmuls into not-yet-started psT
            # banks (g0's start=True overwrites). The PE downclocks to
            # 1.2GHz when idle; these bridge the projections->g0 gap so
            # the score matmuls run at 2.4GHz.
            for w in range(8):
                nc.tensor.matmul(psT[w % 2][:, 0:256], wqk_sb[:, 0:128],
                                 kT_sb[:, 0:256], start=True, stop=True)
            for kt in range(NKC):
                nc.tensor.matmul(psT[kt][:], ksb[0][:, kt * 128:(kt + 1) * 128],
                                 sqb[0][:], start=True, stop=False)
            for w in range(3):
                nc.tensor.matmul(psT[7][:, 0:256], zeros16[:],
                                 sqb[0][:, 0:256], start=False, stop=False)
            for kt in range(NKC):
                nc.tensor.matmul(psT[kt][:], ksb[1][:, kt * 128:(kt + 1) * 128],
                                 sqb[1][:], start=False, stop=False)
            for w in range(5):
                nc.tensor.matmul(psT[0][:, 0:256], zeros16[:],
                                 sqb[0][:, 0:256], start=False, stop=False)
            for kt in range(NKC):
                nc.tensor.matmul(psT[kt][:],
                                 kspair[:, :, kt * 128:(kt + 1) * 128],
                                 sqpair[:], start=False, stop=True,
                                 perf_mode=DR)

            # ---- exp + denominators + output matmuls ----
            # First N_DVE_EXP tiles via the Schraudolph bf16 bit trick on
            # DVE (covers the ScalarE act-table reload); rest ScalarE Exp.
            I16 = mybir.dt.int16
            expT = []
            for kt in range(NKC):
                et = sm.tile([128, 512], BF16, tag="expT", name=f"expT_{kt}")
                if kt < N_DVE_EXP:
                    tf = work.tile([128, 512], F32, tag="schr", bufs=2,
                                   name=f"schr_{kt}")
                    # bits = s*2^7/ln2 + (127 - c)*2^7  (c~0.0455 incl
                    # trunc-vs-round slack), then f32->i16 convert writes
                    # the bf16 pattern of e^s.
                    nc.vector.tensor_scalar(tf[:], psT[kt][:],
                                            184.6650, 16250.18,
                                            AluOpType.mult, AluOpType.add)
                    nc.vector.tensor_copy(et[:].bitcast(I16), tf[:])
                else:
                    nc.scalar.activation(et[:], psT[kt][:], A.Exp)
                expT.append(et)

            ps_outT = ps.tile([128, 512], F32, tag="t512", name="ps_outT")
            psum_sums = ps.tile([1, 512], F32, tag="t512", name="psum_sums")
            for kt in range(NKC):
                nc.tensor.matmul(ps_outT[:], v_sb[:, kt, :], expT[kt][:],
                                 start=(kt == 0), stop=(kt == NKC - 1))
                nc.tensor.matmul(psum_sums[:], ones16[:], expT[kt][:],
                                 start=(kt == 0), stop=(kt == NKC - 1))
            sums_sb = sm.tile([1, 512], F32, tag="sums_sb")
            nc.scalar.activation(sums_sb[:], psum_sums[:], A.Copy)
            nc.gpsimd.dma_start(sums[:, :], sums_sb[:])
            outT_sb = sm.tile([128, 512], BF16, tag="outT_sb")
            nc.vector.tensor_copy(outT_sb[:, 0:256], ps_outT[:, 0:256])
            nc.sync.dma_start(outT[:, 0:256], outT_sb[:, 0:256])
            nc.vector.tensor_copy(outT_sb[:, 256:512], ps_outT[:, 256:512])
            nc.scalar.dma_start(outT[:, 256:512], outT_sb[:, 256:512])

    nc.finalize()
    return nc


def _get_nc():
    if "nc" not in _CACHED:
        _CACHED["nc"] = _build()
    return _CACHED["nc"]


def _make_consts(W_q, W_k, w_v):
    import ml_dtypes
    # wqk layout: [:, 0:128] = [W_q | W_q], [:, 128:256] = [W_k | W_k]
    wqk = np.zeros((128, 256), np.float32)
    wqk[:, 0:64] = W_q
    wqk[:, 64:128] = W_q
    wqk[:, 128:192] = W_k
    wqk[:, 192:256] = W_k
    cvec = np.zeros((128, 16), np.float32)
    # wrap-phase consts (turns, +36 so z lands in [32, 64)):
    # Q packing [sin | cos], K packing [cos | sin]
    cvec[:64, 0] = 36.0
    cvec[64:, 0] = 36.25   # sphq
    cvec[:64, 1] = 36.25
    cvec[64:, 1] = 36.0    # sphk
    cvec[64:, 2] = HALF_PI  # biasq (radians, j=0 direct)
    cvec[:64, 3] = HALF_PI  # biask
    cvec[:, 4] = 65 * np.pi  # bias65
    for j in range(J):
        cwj = (FOURIER_C[j] * w_v).astype(np.float32)
        cvec[:64, 5 + j] = cwj
        cvec[64:, 5 + j] = cwj
    return wqk.astype(ml_dtypes.bfloat16), cvec


def kernel(queries, keys, values, W_q, W_k, w_v, _trace=False, _trace_kwargs=None):
    import ml_dtypes
    from concourse.bass_utils import run_bass_kernel_spmd

    BF = ml_dtypes.bfloat16
    nc = _get_nc()
    wqk, cvec = _make_consts(
        np.asarray(W_q), np.asarray(W_k), np.asarray(w_v))
    queries = np.asarray(queries, np.float32)
    keys = np.asarray(keys, np.float32)
    values = np.asarray(values, np.float32)

    in_maps = []
    for c in range(NCORES):
        b, qh = c // 2, c % 2
        qTa = np.ascontiguousarray(
            queries[b, qh * NQH:(qh + 1) * NQH, :].T).astype(BF)
        kTa = np.ascontiguousarray(keys[b].T).astype(BF)
        # v4[p, ct, :] = values[b][ct*128 + p, :]
        v4a = np.ascontiguousarray(
            values[b].reshape(8, 128, 128).transpose(1, 0, 2)).astype(BF)
        in_maps.append({"qT": qTa, "kT": kTa, "v4": v4a,
                        "wqk": wqk, "cvec": cvec})

    kwargs = {}
    if _trace:
        kwargs["trace"] = True
        kwargs.update(_trace_kwargs or {})
    res = run_bass_kernel_spmd(nc, in_maps, core_ids=list(range(NCORES)), **kwargs)

    out = np.empty((BS, NQ, VD), np.float32)
    for c in range(NCORES):
        b, qh = c // 2, c % 2
        oT = np.asarray(res.results[c]["outT"]).astype(np.float32)  # [vd, q]
        s = np.asarray(res.results[c]["sums"])                       # [1, q]
        out[b, qh * NQH:(qh + 1) * NQH, :] = (oT / s).T
    if _trace:
        return out, res
    return out
